# revision 3
# baseline (speedup 1.0000x reference)
"""DCGRU cell (Chebyshev graph diffusion GRU) on 8 Trainium2 NeuronCores.

Sharding: data-parallel over batch B=32 -> 4 batches/core; support + weights
replicated. Zero collectives.

Per-core program (Bc=4, N=4096, S=128 feats = 64 xin + 64 h):
  "Flipped" diffusion matmuls: stationary = X node-major 128x128 tiles,
  moving = supportT row-panels streamed from DRAM; the output lands
  feature-major (X_T[c, i]) which is exactly the projection layout, and the
  support stream is large-contiguous. PSUM drains per 512-column i-chunk;
  the Chebyshev correction X2 = 2*S@X1 - X0 and the gate projections are
  fused into the chunk epilogues.

  gconv1: X1T = (S@X0)^T (+PE-transpose back to node-major for step 2);
          step2 fuses: X2T, ru-projection, sigmoid, RHT = r*h (pair-packed),
          u saved, and the xin-half of the c-projection folded into CA.
  gconv2: diffusion of RH columns only (xin columns are covered by CA);
          step2 fuses: X2'T, c-projection B-half + CA, tanh,
          new_h = c + u*(h-c).

Matmul dtype bf16 (support/X/W); PSUM + gate math fp32.
"""
import os
import sys
import numpy as np
import ml_dtypes
from contextlib import ExitStack

for _p in ("/opt/trn_rl_repo", "/root/.axon_site/_ro/trn_rl_repo"):
    if os.path.isdir(_p) and _p not in sys.path:
        sys.path.append(_p)

import concourse.bass as bass  # noqa: E402,F401
import concourse.mybir as mybir  # noqa: E402
import concourse.tile as tile  # noqa: E402
from concourse import bacc  # noqa: E402
from concourse.bass_utils import run_bass_kernel_spmd  # noqa: E402
from concourse.masks import make_identity  # noqa: E402

BF = mybir.dt.bfloat16
F32 = mybir.dt.float32
AF = mybir.ActivationFunctionType
OP = mybir.AluOpType
BFNP = ml_dtypes.bfloat16

N = 4096          # nodes
P = 128           # partitions
JT = N // P       # 32 node tiles
B = 32            # full batch
NCORES = 8
BC = B // NCORES  # 4 batches per core
NPR = BC // 2     # 2 batch pairs
D = 64            # input feats
U = 64            # hidden units
S = D + U         # 128
C1 = BC * S       # 512 x0 columns
CT1 = C1 // P     # 4 c-tiles (one per batch)
C2 = BC * U       # 256 rh columns
CT2 = C2 // P     # 2 c-tiles (one per batch pair)
IC = 512          # i-chunk width (psum drain granularity)
NIC = N // IC     # 8
JG = 4            # j-tiles per support panel DMA (512KB bf16)

_CACHE = {}
LAST_RESULT = None


def _diffusion_chunk(nc, psum_pool, panel_pool, st4, lhs_fn, nct, ic, tags, pfx):
    """One i-chunk of a flipped diffusion step (accumulate over 32 j-tiles).

    lhs_fn(jj, t) -> (128,128) bf16 stationary AP for c-tile t.
    Returns the per-c-tile psum tiles, un-drained.
    """
    sl = slice(ic * IC, (ic + 1) * IC)
    pss = []
    for t in range(nct):
        pst = psum_pool.tile([P, IC], F32, tag=tags[t], name=f"{pfx}{tags[t]}_{ic}")
        pss.append(pst)
    for jg in range(JT // JG):
        pan = panel_pool.tile([P, JG, IC], BF, tag="pan", name=f"{pfx}pan_{ic}_{jg}")
        nc.sync.dma_start(pan[:], st4[:, jg * JG:(jg + 1) * JG, sl])
        for j4 in range(JG):
            jj = jg * JG + j4
            for t in range(nct):
                nc.tensor.matmul(
                    pss[t][:], lhs_fn(jj, t), pan[:, j4, :],
                    start=(jj == 0), stop=(jj == JT - 1),
                )
    return pss


def _build():
    nc = bacc.Bacc("TRN2", target_bir_lowering=False, debug=False,
                   num_devices=NCORES)
    st = nc.dram_tensor("st", [N, N], BF, kind="ExternalInput").ap()
    x0n = nc.dram_tensor("x0n", [N, C1], BF, kind="ExternalInput").ap()
    x0t = nc.dram_tensor("x0t", [C1, N], BF, kind="ExternalInput").ap()
    h2 = nc.dram_tensor("h2", [P, NPR, N], BF, kind="ExternalInput").ap()
    wru = nc.dram_tensor("wru", [P, 3, 2 * U], BF, kind="ExternalInput").ap()
    wca = nc.dram_tensor("wca", [D, 3, U], BF, kind="ExternalInput").ap()
    wcb = nc.dram_tensor("wcb", [P, 3, U], BF, kind="ExternalInput").ap()
    bru = nc.dram_tensor("bru", [P, 1], F32, kind="ExternalInput").ap()
    bc2 = nc.dram_tensor("bc2", [P, 1], F32, kind="ExternalInput").ap()
    nh = nc.dram_tensor("nh", [P, NPR, N], F32, kind="ExternalOutput").ap()

    st4 = st.rearrange("(jt p) i -> p jt i", p=P)

    with tile.TileContext(nc) as tc, ExitStack() as ctx:
        wpool = ctx.enter_context(tc.tile_pool(name="w", bufs=1))
        big1 = ctx.enter_context(tc.tile_pool(name="big1", bufs=1))
        big2 = ctx.enter_context(tc.tile_pool(name="big2", bufs=1))
        big3 = ctx.enter_context(tc.tile_pool(name="big3", bufs=1))
        big4 = ctx.enter_context(tc.tile_pool(name="big4", bufs=1))
        h2pool = ctx.enter_context(tc.tile_pool(name="h2p", bufs=1))
        rhtpool = ctx.enter_context(tc.tile_pool(name="rhtp", bufs=1))
        capool = ctx.enter_context(tc.tile_pool(name="cap", bufs=1))
        panel_pool = ctx.enter_context(tc.tile_pool(name="pan", bufs=2))
        x2cpool = ctx.enter_context(tc.tile_pool(name="x2c", bufs=2))
        sfpool = ctx.enter_context(tc.tile_pool(name="sf", bufs=2))
        dps = ctx.enter_context(tc.tile_pool(name="dps", bufs=1, space="PSUM"))
        eps = ctx.enter_context(tc.tile_pool(name="eps", bufs=2, space="PSUM"))

        # ---- constants ----
        wru_sb = wpool.tile([P, 3, 2 * U], BF)
        nc.sync.dma_start(wru_sb[:], wru[:])
        wca_sb = wpool.tile([D, 3, U], BF)
        nc.sync.dma_start(wca_sb[:], wca[:])
        wcb_sb = wpool.tile([P, 3, U], BF)
        nc.sync.dma_start(wcb_sb[:], wcb[:])
        bru_sb = wpool.tile([P, 1], F32)
        nc.sync.dma_start(bru_sb[:], bru[:])
        bc2_sb = wpool.tile([P, 1], F32)
        nc.sync.dma_start(bc2_sb[:], bc2[:])
        ident = wpool.tile([P, P], BF)
        make_identity(nc, ident[:])

        # ---- big tensors (slots reused across phases via tags) ----
        x0n_sb = big1.tile([P, JT, C1], BF, tag="A")   # 32KB/part
        x0n4 = x0n.rearrange("(jt p) c -> p jt c", p=P)
        for g in range(4):
            nc.sync.dma_start(x0n_sb[:, g * 8:(g + 1) * 8, :],
                              x0n4[:, g * 8:(g + 1) * 8, :])
        x1n_sb = big2.tile([P, JT, C1], BF, tag="B")   # X1 node-major
        x0t_sb = big3.tile([P, CT1, N], BF, tag="C")
        x0t4 = x0t.rearrange("(ct p) i -> p ct i", p=P)
        for t in range(CT1):
            nc.sync.dma_start(x0t_sb[:, t, :], x0t4[:, t, :])
        x1t_sb = big4.tile([P, CT1, N], BF, tag="D")
        h2_sb = h2pool.tile([P, NPR, N], BF)
        nc.sync.dma_start(h2_sb[:], h2[:])
        rht_sb = rhtpool.tile([P, NPR, N], BF)         # r*h pair-packed
        ca_sb = capool.tile([P, NPR, N], BF)           # folded xin c-proj

        dtags1 = [f"d{t}" for t in range(CT1)]

        # ================= gconv1 step 1: X1 = S @ X0 =================
        for ic in range(NIC):
            sl = slice(ic * IC, (ic + 1) * IC)
            pss = _diffusion_chunk(
                nc, dps, panel_pool, st4,
                lambda jj, t: x0n_sb[:, jj, t * P:(t + 1) * P],
                CT1, ic, dtags1, "s1")
            for t in range(CT1):
                nc.vector.tensor_copy(x1t_sb[:, t, sl], pss[t][:])
            for t in range(CT1):
                for blk in range(IC // P):
                    tp = eps.tile([P, IC], BF, tag="e0", name=f"tp1_{ic}_{t}_{blk}")
                    nc.tensor.transpose(
                        tp[:, 0:P],
                        x1t_sb[:, t, ic * IC + blk * P:ic * IC + (blk + 1) * P],
                        ident[:])
                    nc.vector.tensor_copy(
                        x1n_sb[:, ic * (IC // P) + blk, t * P:(t + 1) * P],
                        tp[:, 0:P])

        # ====== gconv1 step 2 + ru-proj + gates + CA fold ======
        u_sb = big1.tile([P, NPR, N], F32, tag="A")    # reuses x0n slot
        for ic in range(NIC):
            sl = slice(ic * IC, (ic + 1) * IC)
            pss = _diffusion_chunk(
                nc, dps, panel_pool, st4,
                lambda jj, t: x1n_sb[:, jj, t * P:(t + 1) * P],
                CT1, ic, dtags1, "s2")
            for pr in range(NPR):
                psca = eps.tile([P, IC], F32, tag="e1", name=f"ca_{ic}_{pr}")
                rc = sfpool.tile([P, IC], F32, tag="rc", name=f"rc_{ic}_{pr}")
                for half in range(2):
                    t = pr * 2 + half  # batch index within core
                    x2c = x2cpool.tile([P, IC], BF, tag="x2c", name=f"x2c_{ic}_{t}")
                    nc.vector.scalar_tensor_tensor(
                        x2c[:], pss[t][:], 2.0, x0t_sb[:, t, sl],
                        op0=OP.mult, op1=OP.subtract)
                    psru = eps.tile([P, IC], F32, tag="e0", name=f"ru_{ic}_{t}")
                    nc.tensor.matmul(psru[:], wru_sb[:, 0, :], x0t_sb[:, t, sl],
                                     start=True, stop=False)
                    nc.tensor.matmul(psru[:], wru_sb[:, 1, :], x1t_sb[:, t, sl],
                                     start=False, stop=False)
                    nc.tensor.matmul(psru[:], wru_sb[:, 2, :], x2c[:],
                                     start=False, stop=True)
                    for m, rhs in ((0, x0t_sb[0:D, t, sl]),
                                   (1, x1t_sb[0:D, t, sl]),
                                   (2, x2c[0:D, :])):
                        nc.tensor.matmul(
                            psca[half * U:(half + 1) * U, :],
                            wca_sb[:, m, :], rhs,
                            start=(m == 0), stop=(m == 2),
                            tile_position=(0, half * U), skip_group_check=True)
                    nc.scalar.activation(rc[half * U:(half + 1) * U, :],
                                         psru[0:U, :], AF.Sigmoid,
                                         bias=bru_sb[0:U, :])
                    nc.scalar.activation(u_sb[half * U:(half + 1) * U, pr, sl],
                                         psru[U:2 * U, :], AF.Sigmoid,
                                         bias=bru_sb[U:2 * U, :])
                nc.vector.tensor_mul(rht_sb[:, pr, sl], rc[:], h2_sb[:, pr, sl])
                nc.vector.tensor_copy(ca_sb[:, pr, sl], psca[:])

        # ================= gconv2 =================
        # rhcont: [:, :, 0:C2] = RH node-major, [:, :, C2:2*C2] = X1'B node-major
        rhcont = big2.tile([P, JT, 2 * C2], BF, tag="B")   # reuses x1n slot
        x1bt_sb = big3.tile([P, NPR, N], BF, tag="C")      # reuses x0t slot

        # RH -> node-major (PE transposes); interleaves with g2s1 via deps
        for pr in range(NPR):
            for jt in range(JT):
                tp = eps.tile([P, IC], BF, tag="e0", name=f"tpr_{pr}_{jt}")
                nc.tensor.transpose(
                    tp[:, 0:P], rht_sb[:, pr, jt * P:(jt + 1) * P], ident[:])
                nc.vector.tensor_copy(
                    rhcont[:, jt, pr * P:(pr + 1) * P], tp[:, 0:P])

        dtags2 = [f"d{t}" for t in range(CT2)]

        # --------- gconv2 step 1: X1' = S @ RH ---------
        for ic in range(NIC):
            sl = slice(ic * IC, (ic + 1) * IC)
            pss = _diffusion_chunk(
                nc, dps, panel_pool, st4,
                lambda jj, t: rhcont[:, jj, t * P:(t + 1) * P],
                CT2, ic, dtags2, "g1")
            for pr in range(NPR):
                nc.vector.tensor_copy(x1bt_sb[:, pr, sl], pss[pr][:])
            for pr in range(NPR):
                for blk in range(IC // P):
                    tp = eps.tile([P, IC], BF, tag="e0", name=f"tpg_{ic}_{pr}_{blk}")
                    nc.tensor.transpose(
                        tp[:, 0:P],
                        x1bt_sb[:, pr, ic * IC + blk * P:ic * IC + (blk + 1) * P],
                        ident[:])
                    nc.vector.tensor_copy(
                        rhcont[:, ic * (IC // P) + blk,
                               C2 + pr * P:C2 + (pr + 1) * P],
                        tp[:, 0:P])

        # --------- gconv2 step 2 + c-projection + new_h ---------
        for ic in range(NIC):
            sl = slice(ic * IC, (ic + 1) * IC)
            pss = _diffusion_chunk(
                nc, dps, panel_pool, st4,
                lambda jj, t: rhcont[:, jj, C2 + t * P:C2 + (t + 1) * P],
                CT2, ic, dtags2, "g2")
            for pr in range(NPR):
                x2g = x2cpool.tile([P, IC], BF, tag="x2c", name=f"x2g_{ic}_{pr}")
                nc.vector.scalar_tensor_tensor(
                    x2g[:], pss[pr][:], 2.0, rht_sb[:, pr, sl],
                    op0=OP.mult, op1=OP.subtract)
                psc = eps.tile([P, IC], F32, tag="e1", name=f"c_{ic}_{pr}")
                for half in range(2):
                    hs = slice(half * U, (half + 1) * U)
                    for m, rhs in ((0, rht_sb[hs, pr, sl]),
                                   (1, x1bt_sb[hs, pr, sl]),
                                   (2, x2g[hs, :])):
                        nc.tensor.matmul(
                            psc[hs, :], wcb_sb[hs, m, :], rhs,
                            start=(m == 0), stop=(m == 2),
                            tile_position=(half * U, half * U),
                            skip_group_check=True)
                cpre = sfpool.tile([P, IC], F32, tag="cpre", name=f"cp_{ic}_{pr}")
                nc.vector.scalar_tensor_tensor(
                    cpre[:], psc[:], 1.0, ca_sb[:, pr, sl],
                    op0=OP.mult, op1=OP.add)
                cfin = sfpool.tile([P, IC], F32, tag="cfin", name=f"cf_{ic}_{pr}")
                nc.scalar.activation(cfin[:], cpre[:], AF.Tanh, bias=bc2_sb[:])
                t1 = sfpool.tile([P, IC], F32, tag="t1", name=f"t1_{ic}_{pr}")
                nc.vector.tensor_tensor(t1[:], h2_sb[:, pr, sl], cfin[:],
                                        op=OP.subtract)
                nc.vector.tensor_mul(t1[:], u_sb[:, pr, sl], t1[:])
                nc.vector.tensor_add(t1[:], cfin[:], t1[:])
                nc.sync.dma_start(nh[:, pr, sl], t1[:])
    nc.compile()
    return nc


def _get_nc():
    if "nc" not in _CACHE:
        _CACHE["nc"] = _build()
    return _CACHE["nc"]


def _prep_core(inputs_np, hx_np, c):
    """Build per-core input arrays for core c (batches 4c..4c+3)."""
    bs = slice(c * BC, (c + 1) * BC)
    xin = inputs_np[bs].reshape(BC, N, D)
    h = hx_np[bs].reshape(BC, N, U)
    x0 = np.concatenate([xin, h], axis=2)             # (BC, N, S)
    x0n_ = np.ascontiguousarray(
        x0.transpose(1, 0, 2).reshape(N, C1)).astype(BFNP)
    x0t_ = np.ascontiguousarray(
        x0.transpose(0, 2, 1).reshape(C1, N)).astype(BFNP)
    # h2[p, pr, n] = h[2*pr + p//U, n, p%U]
    ht = h.transpose(0, 2, 1)                          # (BC, U, N)
    h2_ = np.ascontiguousarray(
        ht.reshape(NPR, P, N).transpose(1, 0, 2)).astype(BFNP)
    return x0n_, x0t_, h2_


def kernel(inputs, hx, support, W_ru, b_ru, W_c, b_c):
    global LAST_RESULT
    inputs = np.asarray(inputs, dtype=np.float32)
    hx = np.asarray(hx, dtype=np.float32)
    support = np.asarray(support, dtype=np.float32)
    W_ru = np.asarray(W_ru, dtype=np.float32)
    b_ru = np.asarray(b_ru, dtype=np.float32)
    W_c = np.asarray(W_c, dtype=np.float32)
    b_c = np.asarray(b_c, dtype=np.float32)

    nc = _get_nc()

    st_ = np.ascontiguousarray(support.T).astype(BFNP)
    wru_ = np.ascontiguousarray(W_ru.reshape(S, 3, 2 * U)).astype(BFNP)
    wc3 = W_c.reshape(S, 3, U)
    wca_ = np.ascontiguousarray(wc3[0:D]).astype(BFNP)
    wcb_half = wc3[D:S]
    wcb_ = np.ascontiguousarray(
        np.concatenate([wcb_half, wcb_half], axis=0)).astype(BFNP)
    bru_ = b_ru.reshape(P, 1).astype(np.float32)
    bc2_ = np.tile(b_c, 2).reshape(P, 1).astype(np.float32)

    in_maps = []
    for c in range(NCORES):
        x0n_, x0t_, h2_ = _prep_core(inputs, hx, c)
        in_maps.append({
            "st": st_, "x0n": x0n_, "x0t": x0t_, "h2": h2_,
            "wru": wru_, "wca": wca_, "wcb": wcb_,
            "bru": bru_, "bc2": bc2_,
        })

    res = run_bass_kernel_spmd(nc, in_maps, core_ids=list(range(NCORES)))
    LAST_RESULT = res

    out = np.empty((B, N * U), dtype=np.float32)
    for c in range(NCORES):
        nh = res.results[c]["nh"]                      # (P, NPR, N)
        # nh[p, pr, n] -> newh[4c + 2*pr + p//U, n*U + p%U]
        arr = nh.reshape(2, U, NPR, N).transpose(2, 0, 3, 1)  # (pr, bl, N, U)
        out[c * BC:(c + 1) * BC] = arr.reshape(BC, N * U)
    return out


# revision 8
# speedup vs baseline: 1.2192x; 1.2192x over previous
"""DCGRU cell (Chebyshev graph diffusion GRU) on 8 Trainium2 NeuronCores.

Sharding: data-parallel over batch B=32 -> 4 batches/core; support + weights
replicated. Zero collectives.

Per-core program (Bc=4, N=4096, S=128 feats = 64 xin + 64 h):
  "Flipped" diffusion matmuls: stationary = X node-major 128x128 tiles,
  moving = supportT row-panels streamed from DRAM; the output lands
  feature-major (X_T[c, i]) which is exactly the projection layout, and the
  support stream is large-contiguous. PSUM drains per 512-column i-chunk;
  the Chebyshev correction X2 = 2*S@X1 - X0 and the gate projections are
  fused into the chunk epilogues.

  gconv1: X1T = (S@X0)^T (+PE-transpose back to node-major for step 2);
          step2 fuses: X2T, ru-projection, sigmoid, RHT = r*h (pair-packed),
          u saved, and the xin-half of the c-projection folded into CA.
  gconv2: diffusion of RH columns only (xin columns are covered by CA);
          step2 fuses: X2'T, c-projection B-half + CA, tanh,
          new_h = c + u*(h-c).

Matmul dtype bf16 (support/X/W); PSUM + gate math fp32.
"""
import os
import sys
import numpy as np
import ml_dtypes
from contextlib import ExitStack

for _p in ("/opt/trn_rl_repo", "/root/.axon_site/_ro/trn_rl_repo"):
    if os.path.isdir(_p) and _p not in sys.path:
        sys.path.append(_p)

import concourse.bass as bass  # noqa: E402,F401
import concourse.mybir as mybir  # noqa: E402
import concourse.tile as tile  # noqa: E402
from concourse import bacc  # noqa: E402
from concourse.bass_utils import run_bass_kernel_spmd  # noqa: E402
from concourse.masks import make_identity  # noqa: E402

BF = mybir.dt.bfloat16
F32 = mybir.dt.float32
AF = mybir.ActivationFunctionType
OP = mybir.AluOpType
BFNP = ml_dtypes.bfloat16

N = 4096          # nodes
P = 128           # partitions
JT = N // P       # 32 node tiles
B = 32            # full batch
NCORES = 8
BC = B // NCORES  # 4 batches per core
NPR = BC // 2     # 2 batch pairs
D = 64            # input feats
U = 64            # hidden units
S = D + U         # 128
C1 = BC * S       # 512 x0 columns
CT1 = C1 // P     # 4 c-tiles (one per batch)
C2 = BC * U       # 256 rh columns
CT2 = C2 // P     # 2 c-tiles (one per batch pair)
IC = 512          # i-chunk width (psum drain granularity)
NIC = N // IC     # 8
JG = 4            # j-tiles per support panel DMA (512KB bf16)

_CACHE = {}
LAST_RESULT = None


def _diffusion_chunk(nc, psum_pool, panel_pool, st4, lhs_fn, nct, ic, tags, pfx,
                     pre_jg=None):
    """One i-chunk of a flipped diffusion step (accumulate over 32 j-tiles).

    lhs_fn(jj, t) -> (128,128) bf16 stationary AP for c-tile t.
    Returns the per-c-tile psum tiles, un-drained.
    """
    sl = slice(ic * IC, (ic + 1) * IC)
    pss = []
    for t in range(nct):
        pst = psum_pool.tile([P, IC], F32, tag=tags[t], name=f"{pfx}{tags[t]}_{ic}")
        pss.append(pst)
    for jg in range(JT // JG):
        if pre_jg is not None:
            pre_jg(jg)
        pan = panel_pool.tile([P, JG, IC], BF, tag="pan", name=f"{pfx}pan_{ic}_{jg}")
        nc.sync.dma_start(pan[:], st4[:, jg * JG:(jg + 1) * JG, sl])
        for j4 in range(JG):
            jj = jg * JG + j4
            for t in range(nct):
                nc.tensor.matmul(
                    pss[t][:], lhs_fn(jj, t), pan[:, j4, :],
                    start=(jj == 0), stop=(jj == JT - 1),
                )
    return pss


def _build():
    nc = bacc.Bacc("TRN2", target_bir_lowering=False, debug=False,
                   num_devices=NCORES)
    st = nc.dram_tensor("st", [N, N], BF, kind="ExternalInput").ap()
    x0n = nc.dram_tensor("x0n", [N, C1], BF, kind="ExternalInput").ap()
    x0t = nc.dram_tensor("x0t", [C1, N], BF, kind="ExternalInput").ap()
    h2 = nc.dram_tensor("h2", [P, NPR, N], BF, kind="ExternalInput").ap()
    wru = nc.dram_tensor("wru", [P, 3, 2 * U], BF, kind="ExternalInput").ap()
    wca = nc.dram_tensor("wca", [D, 3, U], BF, kind="ExternalInput").ap()
    wcb = nc.dram_tensor("wcb", [P, 3, U], BF, kind="ExternalInput").ap()
    bru = nc.dram_tensor("bru", [P, 1], F32, kind="ExternalInput").ap()
    bc2 = nc.dram_tensor("bc2", [P, 1], F32, kind="ExternalInput").ap()
    nh = nc.dram_tensor("nh", [P, NPR, N], F32, kind="ExternalOutput").ap()

    st4 = st.rearrange("(jt p) i -> p jt i", p=P)

    with tile.TileContext(nc) as tc, ExitStack() as ctx:
        wpool = ctx.enter_context(tc.tile_pool(name="w", bufs=1))
        big1 = ctx.enter_context(tc.tile_pool(name="big1", bufs=1))
        big2 = ctx.enter_context(tc.tile_pool(name="big2", bufs=1))
        big3 = ctx.enter_context(tc.tile_pool(name="big3", bufs=1))
        h2pool = ctx.enter_context(tc.tile_pool(name="h2p", bufs=1))
        rhtpool = ctx.enter_context(tc.tile_pool(name="rhtp", bufs=1))
        capool = ctx.enter_context(tc.tile_pool(name="cap", bufs=1))
        x2cpool = ctx.enter_context(tc.tile_pool(name="x2c", bufs=2))
        sfpool = ctx.enter_context(tc.tile_pool(name="sf", bufs=2))
        eps = ctx.enter_context(tc.tile_pool(name="eps", bufs=2, space="PSUM"))
        g1stack = ExitStack()
        big4 = g1stack.enter_context(tc.tile_pool(name="big4", bufs=1))
        pan1 = g1stack.enter_context(tc.tile_pool(name="pan1", bufs=2))
        dps1 = g1stack.enter_context(tc.tile_pool(name="dps1", bufs=1, space="PSUM"))

        # ---- constants ----
        wru_sb = wpool.tile([P, 3, 2 * U], BF)
        wca_sb = wpool.tile([D, 3, U], BF)
        wcb_sb = wpool.tile([P, 3, U], BF)
        bru_sb = wpool.tile([P, 1], F32)
        bc2_sb = wpool.tile([P, 1], F32)
        ident = wpool.tile([P, P], BF)
        make_identity(nc, ident[:])

        # ---- big tensors (slots reused across phases via tags) ----
        x0n_sb = big1.tile([P, JT, C1], BF, tag="A")   # 32KB/part
        x0n4 = x0n.rearrange("(jt p) c -> p jt c", p=P)
        x1n_sb = big2.tile([P, JT, C1], BF, tag="B")   # X1 node-major
        x0t_sb = big3.tile([P, CT1, N], BF, tag="C")
        x0t4 = x0t.rearrange("(ct p) i -> p ct i", p=P)
        x1t_sb = big4.tile([P, CT1, N], BF, tag="D")
        h2_sb = h2pool.tile([P, NPR, N], BF)
        rht_sb = rhtpool.tile([P, NPR, N], BF)         # r*h pair-packed
        ca_sb = capool.tile([P, NPR, N], BF)           # folded xin c-proj

        dtags1 = [f"d{t}" for t in range(CT1)]

        # ================= gconv1 step 1: X1 = S @ X0 =================
        for ic in range(NIC):
            sl = slice(ic * IC, (ic + 1) * IC)
            pre = None
            if ic == 0:
                def pre(jg):
                    nc.sync.dma_start(x0n_sb[:, jg * 4:(jg + 1) * 4, :],
                                      x0n4[:, jg * 4:(jg + 1) * 4, :])
            pss = _diffusion_chunk(
                nc, dps1, pan1, st4,
                lambda jj, t: x0n_sb[:, jj, t * P:(t + 1) * P],
                CT1, ic, dtags1, "s1", pre_jg=pre)
            if 1 <= ic <= 4:
                nc.sync.dma_start(x0t_sb[:, ic - 1, :], x0t4[:, ic - 1, :])
            elif ic == 5:
                nc.sync.dma_start(h2_sb[:], h2[:])
            elif ic == 6:
                nc.sync.dma_start(wru_sb[:], wru[:])
                nc.sync.dma_start(wca_sb[:], wca[:])
                nc.sync.dma_start(wcb_sb[:], wcb[:])
                nc.sync.dma_start(bru_sb[:], bru[:])
                nc.sync.dma_start(bc2_sb[:], bc2[:])
            for t in range(CT1):
                nc.vector.tensor_copy(x1t_sb[:, t, sl], pss[t][:])
            for t in range(CT1):
                for blk in range(IC // P):
                    tp = eps.tile([P, IC], F32, tag="e0", name=f"tp1_{ic}_{t}_{blk}")
                    nc.tensor.matmul(
                        tp[:, 0:P],
                        x1t_sb[:, t, ic * IC + blk * P:ic * IC + (blk + 1) * P],
                        ident[:], start=True, stop=True)
                    nc.vector.tensor_copy(
                        x1n_sb[:, ic * (IC // P) + blk, t * P:(t + 1) * P],
                        tp[:, 0:P])

        # ====== gconv1 step 2 + ru-proj + gates + CA fold ======
        u_sb = big1.tile([P, NPR, N], F32, tag="A")    # reuses x0n slot
        for ic in range(NIC):
            sl = slice(ic * IC, (ic + 1) * IC)
            pss = _diffusion_chunk(
                nc, dps1, pan1, st4,
                lambda jj, t: x1n_sb[:, jj, t * P:(t + 1) * P],
                CT1, ic, dtags1, "s2")
            for pr in range(NPR):
                psca = eps.tile([P, IC], F32, tag="e1", name=f"ca_{ic}_{pr}")
                rc = sfpool.tile([P, IC], F32, tag="rc", name=f"rc_{ic}_{pr}")
                for half in range(2):
                    t = pr * 2 + half  # batch index within core
                    x2c = x2cpool.tile([P, IC], BF, tag="x2c", name=f"x2c_{ic}_{t}")
                    nc.vector.scalar_tensor_tensor(
                        x2c[:], pss[t][:], 2.0, x0t_sb[:, t, sl],
                        op0=OP.mult, op1=OP.subtract)
                    psru = eps.tile([P, IC], F32, tag="e0", name=f"ru_{ic}_{t}")
                    nc.tensor.matmul(psru[:], wru_sb[:, 0, :], x0t_sb[:, t, sl],
                                     start=True, stop=False)
                    nc.tensor.matmul(psru[:], wru_sb[:, 1, :], x1t_sb[:, t, sl],
                                     start=False, stop=False)
                    nc.tensor.matmul(psru[:], wru_sb[:, 2, :], x2c[:],
                                     start=False, stop=True)
                    for m, rhs in ((0, x0t_sb[0:D, t, sl]),
                                   (1, x1t_sb[0:D, t, sl]),
                                   (2, x2c[0:D, :])):
                        nc.tensor.matmul(
                            psca[half * U:(half + 1) * U, :],
                            wca_sb[:, m, :], rhs,
                            start=(m == 0), stop=(m == 2),
                            tile_position=(0, half * U), skip_group_check=True)
                    nc.scalar.activation(rc[half * U:(half + 1) * U, :],
                                         psru[0:U, :], AF.Sigmoid,
                                         bias=bru_sb[0:U, :])
                    nc.scalar.activation(u_sb[half * U:(half + 1) * U, pr, sl],
                                         psru[U:2 * U, :], AF.Sigmoid,
                                         bias=bru_sb[U:2 * U, :])
                nc.vector.tensor_mul(rht_sb[:, pr, sl], rc[:], h2_sb[:, pr, sl])
                nc.vector.tensor_copy(ca_sb[:, pr, sl], psca[:])

        # ================= gconv2 =================
        g1stack.close()
        g2stack = ExitStack()
        pan2 = g2stack.enter_context(tc.tile_pool(name="pan2", bufs=4))
        dps2 = g2stack.enter_context(tc.tile_pool(name="dps2", bufs=2, space="PSUM"))
        # rhcont: [:, :, 0:C2] = RH node-major, [:, :, C2:2*C2] = X1'B node-major
        rhcont = big2.tile([P, JT, 2 * C2], BF, tag="B")   # reuses x1n slot
        x1bt_sb = big3.tile([P, NPR, N], BF, tag="C")      # reuses x0t slot

        # RH -> node-major (PE transposes); interleaves with g2s1 via deps
        for pr in range(NPR):
            for jt in range(JT):
                tp = eps.tile([P, IC], F32, tag="e0", name=f"tpr_{pr}_{jt}")
                nc.tensor.matmul(
                    tp[:, 0:P], rht_sb[:, pr, jt * P:(jt + 1) * P], ident[:],
                    start=True, stop=True)
                nc.vector.tensor_copy(
                    rhcont[:, jt, pr * P:(pr + 1) * P], tp[:, 0:P])

        dtags2 = [f"g{t}" for t in range(CT2)]

        # --------- gconv2 step 1: X1' = S @ RH ---------
        for ic in range(NIC):
            sl = slice(ic * IC, (ic + 1) * IC)
            pss = _diffusion_chunk(
                nc, dps2, pan2, st4,
                lambda jj, t: rhcont[:, jj, t * P:(t + 1) * P],
                CT2, ic, dtags2, "g1")
            for pr in range(NPR):
                nc.vector.tensor_copy(x1bt_sb[:, pr, sl], pss[pr][:])
            for pr in range(NPR):
                for blk in range(IC // P):
                    tp = eps.tile([P, IC], F32, tag="e0", name=f"tpg_{ic}_{pr}_{blk}")
                    nc.tensor.matmul(
                        tp[:, 0:P],
                        x1bt_sb[:, pr, ic * IC + blk * P:ic * IC + (blk + 1) * P],
                        ident[:], start=True, stop=True)
                    nc.vector.tensor_copy(
                        rhcont[:, ic * (IC // P) + blk,
                               C2 + pr * P:C2 + (pr + 1) * P],
                        tp[:, 0:P])

        # --------- gconv2 step 2 + c-projection + new_h ---------
        for ic in range(NIC):
            sl = slice(ic * IC, (ic + 1) * IC)
            pss = _diffusion_chunk(
                nc, dps2, pan2, st4,
                lambda jj, t: rhcont[:, jj, C2 + t * P:C2 + (t + 1) * P],
                CT2, ic, dtags2, "g2")
            for pr in range(NPR):
                x2g = x2cpool.tile([P, IC], BF, tag="x2c", name=f"x2g_{ic}_{pr}")
                nc.vector.scalar_tensor_tensor(
                    x2g[:], pss[pr][:], 2.0, rht_sb[:, pr, sl],
                    op0=OP.mult, op1=OP.subtract)
                psc = eps.tile([P, IC], F32, tag="e1", name=f"c_{ic}_{pr}")
                for half in range(2):
                    hs = slice(half * U, (half + 1) * U)
                    for m, rhs in ((0, rht_sb[hs, pr, sl]),
                                   (1, x1bt_sb[hs, pr, sl]),
                                   (2, x2g[hs, :])):
                        nc.tensor.matmul(
                            psc[hs, :], wcb_sb[hs, m, :], rhs,
                            start=(m == 0), stop=(m == 2),
                            tile_position=(half * U, half * U),
                            skip_group_check=True)
                cpre = sfpool.tile([P, IC], F32, tag="cpre", name=f"cp_{ic}_{pr}")
                nc.vector.scalar_tensor_tensor(
                    cpre[:], psc[:], 1.0, ca_sb[:, pr, sl],
                    op0=OP.mult, op1=OP.add)
                cfin = sfpool.tile([P, IC], F32, tag="cfin", name=f"cf_{ic}_{pr}")
                nc.scalar.activation(cfin[:], cpre[:], AF.Tanh, bias=bc2_sb[:])
                t1 = sfpool.tile([P, IC], F32, tag="t1", name=f"t1_{ic}_{pr}")
                nc.vector.tensor_tensor(t1[:], h2_sb[:, pr, sl], cfin[:],
                                        op=OP.subtract)
                nc.vector.tensor_mul(t1[:], u_sb[:, pr, sl], t1[:])
                nc.vector.tensor_add(t1[:], cfin[:], t1[:])
                nc.sync.dma_start(nh[:, pr, sl], t1[:])
        g2stack.close()
    nc.compile()
    return nc


def _get_nc():
    if "nc" not in _CACHE:
        _CACHE["nc"] = _build()
    return _CACHE["nc"]


def _prep_core(inputs_np, hx_np, c):
    """Build per-core input arrays for core c (batches 4c..4c+3)."""
    bs = slice(c * BC, (c + 1) * BC)
    xin = inputs_np[bs].reshape(BC, N, D)
    h = hx_np[bs].reshape(BC, N, U)
    x0 = np.concatenate([xin, h], axis=2)             # (BC, N, S)
    x0n_ = np.ascontiguousarray(
        x0.transpose(1, 0, 2).reshape(N, C1)).astype(BFNP)
    x0t_ = np.ascontiguousarray(
        x0.transpose(0, 2, 1).reshape(C1, N)).astype(BFNP)
    # h2[p, pr, n] = h[2*pr + p//U, n, p%U]
    ht = h.transpose(0, 2, 1)                          # (BC, U, N)
    h2_ = np.ascontiguousarray(
        ht.reshape(NPR, P, N).transpose(1, 0, 2)).astype(BFNP)
    return x0n_, x0t_, h2_


def kernel(inputs, hx, support, W_ru, b_ru, W_c, b_c):
    global LAST_RESULT
    inputs = np.asarray(inputs, dtype=np.float32)
    hx = np.asarray(hx, dtype=np.float32)
    support = np.asarray(support, dtype=np.float32)
    W_ru = np.asarray(W_ru, dtype=np.float32)
    b_ru = np.asarray(b_ru, dtype=np.float32)
    W_c = np.asarray(W_c, dtype=np.float32)
    b_c = np.asarray(b_c, dtype=np.float32)

    nc = _get_nc()

    st_ = np.ascontiguousarray(support.T).astype(BFNP)
    wru_ = np.ascontiguousarray(W_ru.reshape(S, 3, 2 * U)).astype(BFNP)
    wc3 = W_c.reshape(S, 3, U)
    wca_ = np.ascontiguousarray(wc3[0:D]).astype(BFNP)
    wcb_half = wc3[D:S]
    wcb_ = np.ascontiguousarray(
        np.concatenate([wcb_half, wcb_half], axis=0)).astype(BFNP)
    bru_ = b_ru.reshape(P, 1).astype(np.float32)
    bc2_ = np.tile(b_c, 2).reshape(P, 1).astype(np.float32)

    in_maps = []
    for c in range(NCORES):
        x0n_, x0t_, h2_ = _prep_core(inputs, hx, c)
        in_maps.append({
            "st": st_, "x0n": x0n_, "x0t": x0t_, "h2": h2_,
            "wru": wru_, "wca": wca_, "wcb": wcb_,
            "bru": bru_, "bc2": bc2_,
        })

    res = run_bass_kernel_spmd(nc, in_maps, core_ids=list(range(NCORES)))
    LAST_RESULT = res

    out = np.empty((B, N * U), dtype=np.float32)
    for c in range(NCORES):
        nh = res.results[c]["nh"]                      # (P, NPR, N)
        # nh[p, pr, n] -> newh[4c + 2*pr + p//U, n*U + p%U]
        arr = nh.reshape(2, U, NPR, N).transpose(2, 0, 3, 1)  # (pr, bl, N, U)
        out[c * BC:(c + 1) * BC] = arr.reshape(BC, N * U)
    return out


# revision 12
# speedup vs baseline: 1.4293x; 1.1724x over previous
"""DCGRU cell (Chebyshev graph diffusion GRU) on 8 Trainium2 NeuronCores.

Sharding: data-parallel over batch B=32 -> 4 batches/core; support + weights
replicated. Zero collectives.

Per-core program (Bc=4, N=4096, S=128 feats = 64 xin + 64 h):
  "Flipped" diffusion matmuls: stationary = X node-major 128x128 tiles,
  moving = supportT row-panels streamed from DRAM; the output lands
  feature-major (X_T[c, i]) which is exactly the projection layout, and the
  support stream is large-contiguous. PSUM drains per 512-column i-chunk;
  the Chebyshev correction X2 = 2*S@X1 - X0 and the gate projections are
  fused into the chunk epilogues.

  gconv1: X1T = (S@X0)^T (+PE-transpose back to node-major for step 2);
          step2 fuses: X2T, ru-projection, sigmoid, RHT = r*h (pair-packed),
          u saved, and the xin-half of the c-projection folded into CA.
  gconv2: diffusion of RH columns only (xin columns are covered by CA);
          step2 fuses: X2'T, c-projection B-half + CA, tanh,
          new_h = c + u*(h-c).

Matmul dtype bf16 (support/X/W); PSUM + gate math fp32.
"""
import os
import sys
import numpy as np
import ml_dtypes
from contextlib import ExitStack

for _p in ("/opt/trn_rl_repo", "/root/.axon_site/_ro/trn_rl_repo"):
    if os.path.isdir(_p) and _p not in sys.path:
        sys.path.append(_p)

import concourse.bass as bass  # noqa: E402,F401
import concourse.mybir as mybir  # noqa: E402
import concourse.tile as tile  # noqa: E402
from concourse import bacc  # noqa: E402
from concourse.bass_utils import run_bass_kernel_spmd  # noqa: E402
from concourse.masks import make_identity  # noqa: E402

BF = mybir.dt.bfloat16
F32 = mybir.dt.float32
AF = mybir.ActivationFunctionType
OP = mybir.AluOpType
BFNP = ml_dtypes.bfloat16

N = 4096          # nodes
P = 128           # partitions
JT = N // P       # 32 node tiles
B = 32            # full batch
NCORES = 8
BC = B // NCORES  # 4 batches per core
NPR = BC // 2     # 2 batch pairs
D = 64            # input feats
U = 64            # hidden units
S = D + U         # 128
C1 = BC * S       # 512 x0 columns
CT1 = C1 // P     # 4 c-tiles (one per batch)
C2 = BC * U       # 256 rh columns
CT2 = C2 // P     # 2 c-tiles (one per batch pair)
IC = 512          # i-chunk width (psum drain granularity)
NIC = N // IC     # 8
JG = 4            # j-tiles per support panel DMA (512KB bf16)

_CACHE = {}
LAST_RESULT = None


def _diffusion_chunk(nc, psum_pool, panel_pool, st4, lhs_fn, nct, ic, tags, pfx,
                     pre_jg=None):
    """One i-chunk of a flipped diffusion step (accumulate over 32 j-tiles).

    lhs_fn(jj, t) -> (128,128) bf16 stationary AP for c-tile t.
    Returns the per-c-tile psum tiles, un-drained.
    """
    sl = slice(ic * IC, (ic + 1) * IC)
    pss = []
    for t in range(nct):
        pst = psum_pool.tile([P, IC], F32, tag=tags[t], name=f"{pfx}{tags[t]}_{ic}")
        pss.append(pst)
    for jg in range(JT // JG):
        if pre_jg is not None:
            pre_jg(jg)
        pan = panel_pool.tile([P, JG, IC], BF, tag="pan", name=f"{pfx}pan_{ic}_{jg}")
        nc.sync.dma_start(pan[:], st4[:, jg * JG:(jg + 1) * JG, sl])
        for j4 in range(JG):
            jj = jg * JG + j4
            for t in range(nct):
                nc.tensor.matmul(
                    pss[t][:], lhs_fn(jj, t), pan[:, j4, :],
                    start=(jj == 0), stop=(jj == JT - 1),
                )
    return pss


def _build():
    nc = bacc.Bacc("TRN2", target_bir_lowering=False, debug=False,
                   num_devices=NCORES)
    st = nc.dram_tensor("st", [N, N], BF, kind="ExternalInput").ap()
    x0n = nc.dram_tensor("x0n", [N, C1], BF, kind="ExternalInput").ap()
    x0t = nc.dram_tensor("x0t", [C1, N], BF, kind="ExternalInput").ap()
    h2 = nc.dram_tensor("h2", [P, NPR, N], BF, kind="ExternalInput").ap()
    wru = nc.dram_tensor("wru", [P, 3, 2 * U], BF, kind="ExternalInput").ap()
    wca = nc.dram_tensor("wca", [D, 3, U], BF, kind="ExternalInput").ap()
    wcb = nc.dram_tensor("wcb", [P, 3, U], BF, kind="ExternalInput").ap()
    bru = nc.dram_tensor("bru", [P, 1], F32, kind="ExternalInput").ap()
    bc2 = nc.dram_tensor("bc2", [P, 1], F32, kind="ExternalInput").ap()
    nh = nc.dram_tensor("nh", [P, NPR, N], F32, kind="ExternalOutput").ap()

    st4 = st.rearrange("(jt p) i -> p jt i", p=P)

    with tile.TileContext(nc) as tc, ExitStack() as ctx:
        wpool = ctx.enter_context(tc.tile_pool(name="w", bufs=1))
        big1 = ctx.enter_context(tc.tile_pool(name="big1", bufs=1))
        big2 = ctx.enter_context(tc.tile_pool(name="big2", bufs=1))
        big3 = ctx.enter_context(tc.tile_pool(name="big3", bufs=1))
        h2pool = ctx.enter_context(tc.tile_pool(name="h2p", bufs=1))
        rhtpool = ctx.enter_context(tc.tile_pool(name="rhtp", bufs=1))
        capool = ctx.enter_context(tc.tile_pool(name="cap", bufs=1))
        x2cpool = ctx.enter_context(tc.tile_pool(name="x2c", bufs=2))
        sfpool = ctx.enter_context(tc.tile_pool(name="sf", bufs=2))
        eps = ctx.enter_context(tc.tile_pool(name="eps", bufs=2, space="PSUM"))
        g1stack = ExitStack()
        big4 = g1stack.enter_context(tc.tile_pool(name="big4", bufs=1))
        pan1 = g1stack.enter_context(tc.tile_pool(name="pan1", bufs=3))
        dps1 = g1stack.enter_context(tc.tile_pool(name="dps1", bufs=1, space="PSUM"))

        # ---- constants ----
        wru_sb = wpool.tile([P, 3, 2 * U], BF)
        wca_sb = wpool.tile([D, 3, U], BF)
        wcb_sb = wpool.tile([P, 3, U], BF)
        bru_sb = wpool.tile([P, 1], F32)
        bc2_sb = wpool.tile([P, 1], F32)
        ident = wpool.tile([P, P], BF)
        make_identity(nc, ident[:])

        # ---- big tensors (slots reused across phases via tags) ----
        x0n_sb = big1.tile([P, JT, C1], BF, tag="A")   # 32KB/part
        x0n4 = x0n.rearrange("(jt p) c -> p jt c", p=P)
        x1n_sb = big2.tile([P, JT, C1], BF, tag="B")   # X1 node-major
        x0t_sb = big3.tile([P, CT1, N], BF, tag="C")
        x0t4 = x0t.rearrange("(ct p) i -> p ct i", p=P)
        x1t_sb = big4.tile([P, CT1, N], BF, tag="D")
        h2_sb = h2pool.tile([P, NPR, N], BF)
        rht_sb = rhtpool.tile([P, NPR, N], BF)         # r*h pair-packed
        ca_sb = capool.tile([P, NPR, N], BF)           # folded xin c-proj

        dtags1 = [f"d{t}" for t in range(CT1)]

        # ================= gconv1 step 1: X1 = S @ X0 =================
        for ic in range(NIC):
            sl = slice(ic * IC, (ic + 1) * IC)
            pre = None
            if ic == 0:
                def pre(jg):
                    nc.sync.dma_start(x0n_sb[:, jg * 4:(jg + 1) * 4, :],
                                      x0n4[:, jg * 4:(jg + 1) * 4, :])
            pss = _diffusion_chunk(
                nc, dps1, pan1, st4,
                lambda jj, t: x0n_sb[:, jj, t * P:(t + 1) * P],
                CT1, ic, dtags1, "s1", pre_jg=pre)
            if 1 <= ic <= 4:
                nc.sync.dma_start(x0t_sb[:, ic - 1, :], x0t4[:, ic - 1, :])
            elif ic == 5:
                nc.sync.dma_start(h2_sb[:], h2[:])
            elif ic == 6:
                nc.sync.dma_start(wru_sb[:], wru[:])
                nc.sync.dma_start(wca_sb[:], wca[:])
                nc.sync.dma_start(wcb_sb[:], wcb[:])
                nc.sync.dma_start(bru_sb[:], bru[:])
                nc.sync.dma_start(bc2_sb[:], bc2[:])
            for t in range(CT1):
                nc.vector.tensor_copy(x1t_sb[:, t, sl], pss[t][:])
            for t in range(CT1):
                for blk in range(IC // P):
                    tp = eps.tile([P, IC], F32, tag=f"e{(t * 4 + blk) % 2}",
                                  name=f"tp1_{ic}_{t}_{blk}")
                    nc.tensor.matmul(
                        tp[:, 0:P],
                        x1t_sb[:, t, ic * IC + blk * P:ic * IC + (blk + 1) * P],
                        ident[:], start=True, stop=True)
                    nc.vector.tensor_copy(
                        x1n_sb[:, ic * (IC // P) + blk, t * P:(t + 1) * P],
                        tp[:, 0:P])

        # ====== gconv1 step 2 + ru-proj + gates + CA fold ======
        u_sb = big1.tile([P, NPR, N], F32, tag="A")    # reuses x0n slot
        for ic in range(NIC):
            sl = slice(ic * IC, (ic + 1) * IC)
            pss = _diffusion_chunk(
                nc, dps1, pan1, st4,
                lambda jj, t: x1n_sb[:, jj, t * P:(t + 1) * P],
                CT1, ic, dtags1, "s2")
            for pr in range(NPR):
                psca = eps.tile([P, IC], F32, tag="e1", name=f"ca_{ic}_{pr}")
                rc = sfpool.tile([P, IC], F32, tag="rc", name=f"rc_{ic}_{pr}")
                for half in range(2):
                    t = pr * 2 + half  # batch index within core
                    x2c = x2cpool.tile([P, IC], BF, tag="x2c", name=f"x2c_{ic}_{t}")
                    nc.vector.scalar_tensor_tensor(
                        x2c[:], pss[t][:], 2.0, x0t_sb[:, t, sl],
                        op0=OP.mult, op1=OP.subtract)
                    psru = eps.tile([P, IC], F32, tag="e0", name=f"ru_{ic}_{t}")
                    nc.tensor.matmul(psru[:], wru_sb[:, 0, :], x0t_sb[:, t, sl],
                                     start=True, stop=False)
                    nc.tensor.matmul(psru[:], wru_sb[:, 1, :], x1t_sb[:, t, sl],
                                     start=False, stop=False)
                    nc.tensor.matmul(psru[:], wru_sb[:, 2, :], x2c[:],
                                     start=False, stop=True)
                    for m, rhs in ((0, x0t_sb[0:D, t, sl]),
                                   (1, x1t_sb[0:D, t, sl]),
                                   (2, x2c[0:D, :])):
                        nc.tensor.matmul(
                            psca[half * U:(half + 1) * U, :],
                            wca_sb[:, m, :], rhs,
                            start=(m == 0), stop=(m == 2),
                            tile_position=(0, half * U), skip_group_check=True)
                    nc.scalar.activation(rc[half * U:(half + 1) * U, :],
                                         psru[0:U, :], AF.Sigmoid,
                                         bias=bru_sb[0:U, :])
                    nc.scalar.activation(u_sb[half * U:(half + 1) * U, pr, sl],
                                         psru[U:2 * U, :], AF.Sigmoid,
                                         bias=bru_sb[U:2 * U, :])
                nc.vector.tensor_mul(rht_sb[:, pr, sl], rc[:], h2_sb[:, pr, sl])
                nc.vector.tensor_copy(ca_sb[:, pr, sl], psca[:])

        # ================= gconv2 =================
        g1stack.close()
        g2stack = ExitStack()
        pan2 = g2stack.enter_context(tc.tile_pool(name="pan2", bufs=6))
        dps2 = g2stack.enter_context(tc.tile_pool(name="dps2", bufs=2, space="PSUM"))
        # rhcont: [:, :, 0:C2] = RH node-major, [:, :, C2:2*C2] = X1'B node-major
        rhcont = big2.tile([P, JT, 2 * C2], BF, tag="B")   # reuses x1n slot
        x1bt_sb = big3.tile([P, NPR, N], BF, tag="C")      # reuses x0t slot

        # RH -> node-major (PE transposes); interleaves with g2s1 via deps
        for pr in range(NPR):
            for jt in range(JT):
                tp = eps.tile([P, IC], F32, tag=f"e{jt % 2}",
                              name=f"tpr_{pr}_{jt}")
                nc.tensor.matmul(
                    tp[:, 0:P], rht_sb[:, pr, jt * P:(jt + 1) * P], ident[:],
                    start=True, stop=True)
                nc.vector.tensor_copy(
                    rhcont[:, jt, pr * P:(pr + 1) * P], tp[:, 0:P])

        dtags2 = [f"g{t}" for t in range(CT2)]

        # --------- gconv2 step 1: X1' = S @ RH ---------
        for ic in range(NIC):
            sl = slice(ic * IC, (ic + 1) * IC)
            pss = _diffusion_chunk(
                nc, dps2, pan2, st4,
                lambda jj, t: rhcont[:, jj, t * P:(t + 1) * P],
                CT2, ic, dtags2, "g1")
            for pr in range(NPR):
                nc.vector.tensor_copy(x1bt_sb[:, pr, sl], pss[pr][:])
            for pr in range(NPR):
                for blk in range(IC // P):
                    tp = eps.tile([P, IC], F32, tag=f"e{blk % 2}",
                                  name=f"tpg_{ic}_{pr}_{blk}")
                    nc.tensor.matmul(
                        tp[:, 0:P],
                        x1bt_sb[:, pr, ic * IC + blk * P:ic * IC + (blk + 1) * P],
                        ident[:], start=True, stop=True)
                    nc.vector.tensor_copy(
                        rhcont[:, ic * (IC // P) + blk,
                               C2 + pr * P:C2 + (pr + 1) * P],
                        tp[:, 0:P])

        # --------- gconv2 step 2 + c-projection + new_h ---------
        for ic in range(NIC):
            sl = slice(ic * IC, (ic + 1) * IC)
            pss = _diffusion_chunk(
                nc, dps2, pan2, st4,
                lambda jj, t: rhcont[:, jj, C2 + t * P:C2 + (t + 1) * P],
                CT2, ic, dtags2, "g2")
            for pr in range(NPR):
                x2g = x2cpool.tile([P, IC], BF, tag="x2c", name=f"x2g_{ic}_{pr}")
                nc.vector.scalar_tensor_tensor(
                    x2g[:], pss[pr][:], 2.0, rht_sb[:, pr, sl],
                    op0=OP.mult, op1=OP.subtract)
                psc = eps.tile([P, IC], F32, tag="e1", name=f"c_{ic}_{pr}")
                for half in range(2):
                    hs = slice(half * U, (half + 1) * U)
                    for m, rhs in ((0, rht_sb[hs, pr, sl]),
                                   (1, x1bt_sb[hs, pr, sl]),
                                   (2, x2g[hs, :])):
                        nc.tensor.matmul(
                            psc[hs, :], wcb_sb[hs, m, :], rhs,
                            start=(m == 0), stop=(m == 2),
                            tile_position=(half * U, half * U),
                            skip_group_check=True)
                cpre = sfpool.tile([P, IC], F32, tag="cpre", name=f"cp_{ic}_{pr}")
                nc.vector.scalar_tensor_tensor(
                    cpre[:], psc[:], 1.0, ca_sb[:, pr, sl],
                    op0=OP.mult, op1=OP.add)
                cfin = sfpool.tile([P, IC], F32, tag="cfin", name=f"cf_{ic}_{pr}")
                nc.scalar.activation(cfin[:], cpre[:], AF.Tanh, bias=bc2_sb[:])
                t1 = sfpool.tile([P, IC], F32, tag="t1", name=f"t1_{ic}_{pr}")
                nc.vector.tensor_tensor(t1[:], h2_sb[:, pr, sl], cfin[:],
                                        op=OP.subtract)
                nc.vector.tensor_mul(t1[:], u_sb[:, pr, sl], t1[:])
                nc.vector.tensor_add(t1[:], cfin[:], t1[:])
                nc.sync.dma_start(nh[:, pr, sl], t1[:])
        g2stack.close()
    nc.compile()
    return nc


def _get_nc():
    if "nc" not in _CACHE:
        _CACHE["nc"] = _build()
    return _CACHE["nc"]


def _prep_core(inputs_np, hx_np, c):
    """Build per-core input arrays for core c (batches 4c..4c+3)."""
    bs = slice(c * BC, (c + 1) * BC)
    xin = inputs_np[bs].reshape(BC, N, D)
    h = hx_np[bs].reshape(BC, N, U)
    x0 = np.concatenate([xin, h], axis=2)             # (BC, N, S)
    x0n_ = np.ascontiguousarray(
        x0.transpose(1, 0, 2).reshape(N, C1)).astype(BFNP)
    x0t_ = np.ascontiguousarray(
        x0.transpose(0, 2, 1).reshape(C1, N)).astype(BFNP)
    # h2[p, pr, n] = h[2*pr + p//U, n, p%U]
    ht = h.transpose(0, 2, 1)                          # (BC, U, N)
    h2_ = np.ascontiguousarray(
        ht.reshape(NPR, P, N).transpose(1, 0, 2)).astype(BFNP)
    return x0n_, x0t_, h2_


def kernel(inputs, hx, support, W_ru, b_ru, W_c, b_c):
    global LAST_RESULT
    inputs = np.asarray(inputs, dtype=np.float32)
    hx = np.asarray(hx, dtype=np.float32)
    support = np.asarray(support, dtype=np.float32)
    W_ru = np.asarray(W_ru, dtype=np.float32)
    b_ru = np.asarray(b_ru, dtype=np.float32)
    W_c = np.asarray(W_c, dtype=np.float32)
    b_c = np.asarray(b_c, dtype=np.float32)

    nc = _get_nc()

    st_ = np.ascontiguousarray(support.T).astype(BFNP)
    wru_ = np.ascontiguousarray(W_ru.reshape(S, 3, 2 * U)).astype(BFNP)
    wc3 = W_c.reshape(S, 3, U)
    wca_ = np.ascontiguousarray(wc3[0:D]).astype(BFNP)
    wcb_half = wc3[D:S]
    wcb_ = np.ascontiguousarray(
        np.concatenate([wcb_half, wcb_half], axis=0)).astype(BFNP)
    bru_ = b_ru.reshape(P, 1).astype(np.float32)
    bc2_ = np.tile(b_c, 2).reshape(P, 1).astype(np.float32)

    in_maps = []
    for c in range(NCORES):
        x0n_, x0t_, h2_ = _prep_core(inputs, hx, c)
        in_maps.append({
            "st": st_, "x0n": x0n_, "x0t": x0t_, "h2": h2_,
            "wru": wru_, "wca": wca_, "wcb": wcb_,
            "bru": bru_, "bc2": bc2_,
        })

    res = run_bass_kernel_spmd(nc, in_maps, core_ids=list(range(NCORES)))
    LAST_RESULT = res

    out = np.empty((B, N * U), dtype=np.float32)
    for c in range(NCORES):
        nh = res.results[c]["nh"]                      # (P, NPR, N)
        # nh[p, pr, n] -> newh[4c + 2*pr + p//U, n*U + p%U]
        arr = nh.reshape(2, U, NPR, N).transpose(2, 0, 3, 1)  # (pr, bl, N, U)
        out[c * BC:(c + 1) * BC] = arr.reshape(BC, N * U)
    return out
